# revision 4
# baseline (speedup 1.0000x reference)
"""Grouped GEMM (MoE routing) Trainium2 kernel — token-streaming fp8 design.

Expert-parallel across 8 NeuronCores with size-sorted slot assignment
(slot s on core c holds the expert of size-rank 8*perm[s]+c; per-slot
capacity cap_s = roundup4(max count in rank group)).

Design notes (v2):
- Weights quantized to float8e3 (E3M4) on host with a global power-of-2
  scale folded into x; weight HBM traffic 34.1 MB/core.  x stays bf16 as
  the PE moving operand (mixed-dtype matmul), out written bf16.
- Token-streaming orientation: stationary = w tile [128k x 128n] fp8,
  moving = x^T [128k x cap] bf16, psum [128n x cap].
- 4 DMA queue-types: weights round-robin on sync/scalar/gpsimd HW+SW DGE
  rings, x^T tiles on the vector (DVE) ring, outputs on gpsimd (last two
  slots on sync for low completion latency at kernel exit).
- Slot processing order interleaves big/small caps so the PE never idles
  long enough for the HAM clock governor to halve the PE clock, and the
  kernel tail ends on wide (PE-efficient) matmuls.
- Warmup trimmed to 16 dummy matmuls — just enough to cover the DMA
  pipe-priming window and ramp the clock before slot 0's weights land.
- Kernel semaphore range shrunk (monkeypatch) so the fixed per-semaphore
  clear storms at NEFF entry/exit cover ~58 sems instead of 106.
"""
import ml_dtypes
import numpy as np

import concourse.bass as _bass_mod
import concourse.mybir as mybir
import concourse.tile as tile
from concourse import bacc
from concourse.bass_utils import run_bass_kernel_spmd

# Shrink the bass-managed semaphore pool: the NEFF prologue/epilogue clear
# every semaphore in the kernel range one-by-one (~28-100ns each), so a
# smaller range directly cuts fixed overhead.  The tile scheduler in this
# kernel allocates ~25 live semaphores; 58 leaves ample margin.
_orig_sem_range = _bass_mod.get_kernel_semaphore_range


def _small_sem_range():
    r = _orig_sem_range()
    return range(r.start, min(r.stop, r.start + 58))


_bass_mod.get_kernel_semaphore_range = _small_sem_range

G, T, DIN, DOUT = 64, 8192, 2560, 1664
NCORES = 8
EPC = G // NCORES   # expert slots per core
KC = DIN // 128     # 20 contraction chunks
NN = DOUT // 128    # 13 output-row chunks

# Processing order of size-ranks: interleave big/small so PE:DMA demand
# stays balanced through the kernel and the tail ends on wide matmuls.
PERM = [0, 7, 1, 6, 2, 5, 3, 4]

_cache = {}


def _build(caps):
    caps = [int(c) for c in caps if c > 0]
    offs = np.concatenate([[0], np.cumsum(caps)]).astype(int)
    S = int(offs[-1])
    nc = bacc.Bacc(trn_type="TRN2", debug=False)
    f8 = mybir.dt.float8e3
    bf16 = mybir.dt.bfloat16
    f32 = mybir.dt.float32

    # partition-major layouts: every DMA below is a [128, N] slice whose
    # per-partition bytes are contiguous in HBM (large descriptors)
    w8 = nc.dram_tensor("w8", [EPC, 128, KC * DOUT], f8,
                        kind="ExternalInput").ap()
    xt = nc.dram_tensor("xt", [128, KC * S], bf16, kind="ExternalInput").ap()
    out = nc.dram_tensor("out", [128, NN * S], bf16, kind="ExternalOutput").ap()

    WB = 4   # k-chunks per w DMA
    NWCH = KC // WB  # 5 w chunks per slot
    with tile.TileContext(nc) as tc:
        with (
            tc.tile_pool(name="wp", bufs=3) as w_pool,
            tc.tile_pool(name="xp", bufs=1) as x_pool,
            tc.tile_pool(name="op", bufs=2) as o_pool,
            tc.tile_pool(name="ps", bufs=1, space="PSUM") as ps_pool,
        ):
            # PE warm-up: dummy matmuls on a zeroed tile so the HAM clock
            # gate ramps toward 2.4 GHz while the first DMAs land.
            warm_l = x_pool.tile([128, 128], bf16, tag="wl", name="warm_l")
            warm_r = x_pool.tile([128, 512], bf16, tag="wr", name="warm_r")
            nc.vector.memset(warm_l[:], 0)
            nc.vector.memset(warm_r[:], 0)
            pswarm = ps_pool.tile([128, 512], f32, tag="psw", name="pswarm")
            for i in range(16):
                nc.tensor.matmul(pswarm[:], warm_l[:], warm_r[:],
                                 start=True, stop=True)
            psums = {}
            for j in range(7):  # one open accumulation region per bank
                psums[j] = ps_pool.tile([128, 512], f32, tag=f"ps{j}",
                                        name=f"psum{j}")
            psums[7] = pswarm  # warmup bank doubles as the 8th region

            WQ = (nc.sync, nc.scalar, nc.gpsimd, nc.sync, nc.scalar)
            for s, cap in enumerate(caps):
                off = int(offs[s])
                # per-slot x^T tile, prefetched on the gpsimd ring ahead of
                # that slot's j2 weight chunk (program order = ring order).
                xs = x_pool.tile([128, KC * cap], bf16, tag="xs",
                                 name=f"xs{s}", bufs=3)
                nc.gpsimd.dma_start(xs[:], xt[:, KC * off:KC * (off + cap)])
                # 852KB weight chunks round-robin over sync/scalar/gpsimd;
                # ~13.6/13.6/6.8 MB of weights per ring + outs on gpsimd
                # roughly balances the three rings.
                wch = []
                for j in range(NWCH):
                    wj = w_pool.tile([128, WB * DOUT], f8, tag=f"w{j}",
                                     name=f"w{s}_{j}", bufs=4 if j < 2 else 3)
                    WQ[j].dma_start(
                        wj[:], w8[s, :, j * WB * DOUT:(j + 1) * WB * DOUT]
                    )
                    wch.append(wj)
                o_sb = o_pool.tile([128, NN * cap], bf16, tag="o", name=f"o{s}")
                assert cap <= 256
                # k-outer within each n-phase: weight/xt chunks are consumed
                # progressively; each PSUM bank hosts exactly one open
                # accumulation region at a time (start/stop clear per bank).
                # The last slot gets a finer final phase so the kernel-exit
                # output DMA is tiny.
                if s == len(caps) - 1:
                    phases = ((0, 8), (8, 12), (12, NN))
                else:
                    phases = ((0, 8), (8, NN))
                for n0, n1 in phases:
                    for k in range(KC):
                        wk = wch[k // WB]
                        kb = (k % WB) * DOUT
                        for n in range(n0, n1):
                            ps = psums[n - n0][:, :cap]
                            nc.tensor.matmul(
                                ps,
                                wk[:, kb + n * 128:kb + (n + 1) * 128],
                                xs[:, k * cap:(k + 1) * cap],
                                start=(k == 0),
                                stop=(k == KC - 1),
                            )
                    for n in range(n0, n1):
                        nc.vector.tensor_copy(
                            o_sb[:, n * cap:(n + 1) * cap],
                            psums[n - n0][:, :cap],
                        )
                    # per-phase output DMA shortens the kernel tail; the last
                    # slots' outs ride HWDGE (lower completion latency than
                    # SWDGE on the critical exit path)
                    if s >= len(caps) - 2:
                        oeng = nc.sync
                    else:
                        oeng = nc.scalar if (s + n0) % 2 else nc.gpsimd
                    oeng.dma_start(
                        out[:, NN * off + n0 * cap:NN * off + n1 * cap],
                        o_sb[:, n0 * cap:n1 * cap],
                    )
    nc.compile()
    return nc


def _run(inputs, trace=False):
    x = np.asarray(inputs["input"], dtype=np.float32)
    w = np.asarray(inputs["weight"], dtype=np.float32)
    counts = np.asarray(inputs["tokens_per_expert"], dtype=np.int64)
    starts = np.concatenate([[0], np.cumsum(counts)[:-1]])

    order = np.argsort(-counts, kind="stable")  # experts by size rank
    perm = PERM
    caps = tuple(
        int(np.ceil(max(1, counts[order[r * NCORES:(r + 1) * NCORES]].max()) / 4) * 4)
        for r in perm
    )
    offs = np.concatenate([[0], np.cumsum(caps)]).astype(int)
    S = int(offs[-1])

    if caps not in _cache:
        _cache[caps] = _build(caps)
    nc = _cache[caps]

    # fp8 scale: w*s must fit in e3m4 (max normal 15.5); fold 1/s into x
    s_pow = 2.0 ** np.floor(np.log2(15.49 / np.abs(w).max()))
    x_sc = (x * (1.0 / s_pow)).astype(ml_dtypes.bfloat16)
    w8_full = (w * s_pow).astype(ml_dtypes.float8_e3m4)

    in_maps = []
    for c in range(NCORES):
        xt_pack = np.zeros((128, KC * S), dtype=ml_dtypes.bfloat16)
        w_pack = np.empty((EPC, 128, KC * DOUT), dtype=ml_dtypes.float8_e3m4)
        for s in range(EPC):
            g = int(order[perm[s] * NCORES + c])
            cnt = int(counts[g])
            cap = caps[s]
            o0 = KC * int(offs[s])
            if cnt:
                blk = np.zeros((128, KC, cap), dtype=ml_dtypes.bfloat16)
                blk[:, :, :cnt] = (
                    x_sc[starts[g]:starts[g] + cnt].T
                    .reshape(KC, 128, cnt).transpose(1, 0, 2)
                )
                xt_pack[:, o0:o0 + KC * cap] = blk.reshape(128, KC * cap)
            w_pack[s] = (
                w8_full[g].reshape(KC, 128, DOUT).transpose(1, 0, 2)
                .reshape(128, KC * DOUT)
            )
        in_maps.append({"w8": w_pack, "xt": xt_pack})

    kw = {"trace_cores": list(range(NCORES))} if trace else {}
    res = run_bass_kernel_spmd(nc, in_maps, core_ids=list(range(NCORES)),
                               trace=trace, **kw)

    out = np.empty((T, DOUT), dtype=np.float32)
    for c in range(NCORES):
        ob = res.results[c]["out"]
        for s in range(EPC):
            g = int(order[perm[s] * NCORES + c])
            cnt = int(counts[g])
            cap = caps[s]
            if cnt:
                blk = ob[:, NN * offs[s]:NN * offs[s] + NN * cap]
                blk = blk.reshape(128, NN, cap).transpose(2, 1, 0)
                out[starts[g]:starts[g] + cnt] = (
                    blk.reshape(cap, DOUT)[:cnt].astype(np.float32)
                )
    return out, res


def kernel(**inputs) -> np.ndarray:
    return _run(inputs)[0]


# revision 5
# speedup vs baseline: 1.1121x; 1.1121x over previous
"""Grouped GEMM (MoE routing) Trainium2 kernel — token-streaming fp8 design.

Expert-parallel across 8 NeuronCores with size-sorted slot assignment
(slot s on core c holds the expert of size-rank 8s+c; per-slot capacity
cap_s = roundup4(max count in rank group)).

Key design vs the bf16 token-stationary baseline:
- Weights are quantized to float8e3 (E3M4, 4 mantissa bits) on host with
  a global power-of-2 scale folded into x (y = (x/s) @ (w*s) exactly), so
  weight HBM traffic halves: 68 MB -> 34 MB per core.  x stays bf16 as
  the PE moving operand (mixed-dtype matmul), out written bf16.
- Token-streaming orientation: stationary = w tile [128k x 128n] fp8,
  moving = x^T [128k x cap] bf16, psum [128n x cap].  PE cost scales with
  actual token count instead of ceil(count/128)*128.
- PSUM bank-group rotation: each slot runs 4 n-phases (0,4)(4,8)(8,12)
  (12,13) on alternating bank groups 0-3 / 4-7, so a phase's first matmul
  reuses banks whose psum->sbuf copies completed a full phase earlier —
  no copy-latency bubble at phase/slot boundaries (the bubbles triggered
  the HAM clock governor to halve the PE clock in the small-cap tail).
- Per-phase output DMA keeps the kernel-exit DMA tiny (1-bank phase).
"""
import ml_dtypes
import numpy as np

import concourse.mybir as mybir
import concourse.tile as tile
from concourse import bacc
from concourse.bass_utils import run_bass_kernel_spmd

G, T, DIN, DOUT = 64, 8192, 2560, 1664
NCORES = 8
EPC = G // NCORES   # expert slots per core
KC = DIN // 128     # 20 contraction chunks
NN = DOUT // 128    # 13 output-row chunks

_cache = {}


def _build(caps):
    caps = [int(c) for c in caps if c > 0]
    offs = np.concatenate([[0], np.cumsum(caps)]).astype(int)
    S = int(offs[-1])
    nc = bacc.Bacc(trn_type="TRN2", debug=False)
    f8 = mybir.dt.float8e3
    bf16 = mybir.dt.bfloat16
    f32 = mybir.dt.float32

    # partition-major layouts: every DMA below is a [128, N] slice whose
    # per-partition bytes are contiguous in HBM (large descriptors)
    w8 = nc.dram_tensor("w8", [EPC, 128, KC * DOUT], f8,
                        kind="ExternalInput").ap()
    xt = nc.dram_tensor("xt", [128, KC * S], bf16, kind="ExternalInput").ap()
    out = nc.dram_tensor("out", [128, NN * S], bf16, kind="ExternalOutput").ap()

    WB = 4   # k-chunks per w DMA
    NWCH = KC // WB  # 5 w chunks per slot
    with tile.TileContext(nc) as tc:
        with (
            tc.tile_pool(name="wp", bufs=3) as w_pool,
            tc.tile_pool(name="xp", bufs=1) as x_pool,
            tc.tile_pool(name="op", bufs=2) as o_pool,
            tc.tile_pool(name="ps", bufs=1, space="PSUM") as ps_pool,
        ):
            # PE warm-up: ~6us of dummy matmuls on a zeroed tile so the HAM
            # clock gate reaches 2.4 GHz before the first real matmul, and the
            # PE is busy while the first DMAs land.
            warm_l = x_pool.tile([128, 128], bf16, tag="wl", name="warm_l")
            warm_r = x_pool.tile([128, 512], bf16, tag="wr", name="warm_r")
            nc.vector.memset(warm_l[:], 0)
            nc.vector.memset(warm_r[:], 0)
            pswarm = ps_pool.tile([128, 512], f32, tag="psw", name="pswarm")
            for i in range(28):
                nc.tensor.matmul(pswarm[:], warm_l[:], warm_r[:],
                                 start=True, stop=True)
            psums = {}
            for j in range(7):  # one open accumulation region per bank
                psums[j] = ps_pool.tile([128, 512], f32, tag=f"ps{j}",
                                        name=f"psum{j}")
            psums[7] = pswarm  # warmup bank doubles as the 8th region

            for s, cap in enumerate(caps):
                off = int(offs[s])
                # per-slot x^T tile (1.3MB at cap 256): prefetched like the
                # weights; slots 0/1 ride the fast HWDGE rings so slot 0
                # starts right as the warmup ends, later slots go SWDGE
                xs = x_pool.tile([128, KC * cap], bf16, tag="xs",
                                 name=f"xs{s}", bufs=3)
                if s == 0:
                    # slot 0 is the startup critical path: split x^T across
                    # the two queues that don't carry w0 chunk 0
                    h = (KC // 2) * cap
                    nc.scalar.dma_start(xs[:, :h], xt[:, KC * off:KC * off + h])
                    nc.gpsimd.dma_start(
                        xs[:, h:], xt[:, KC * off + h:KC * (off + cap)]
                    )
                else:
                    xeng = nc.scalar if s == 1 else nc.gpsimd
                    xeng.dma_start(xs[:], xt[:, KC * off:KC * (off + cap)])
                # 852KB weight chunks; queue choreography: slot 0's chunks
                # sequenced so each arrives just before phase A consumes it,
                # tail slots use all three queues (the HWDGE rings alone
                # can't stream the tail while slots shorten)
                WQ = {
                    0: (nc.sync, nc.sync, nc.scalar, nc.sync, nc.scalar),
                    3: (nc.sync, nc.scalar, nc.sync, nc.scalar, nc.gpsimd),
                    4: (nc.sync, nc.scalar, nc.sync, nc.scalar, nc.gpsimd),
                    5: (nc.sync, nc.scalar, nc.sync, nc.scalar, nc.gpsimd),
                    6: (nc.scalar, nc.sync, nc.gpsimd, nc.scalar, nc.sync),
                    7: (nc.sync, nc.scalar, nc.gpsimd, nc.sync, nc.scalar),
                }
                wch = []
                for j in range(NWCH):
                    # tags 0/1 ride the HWDGE rings for every slot, where ring
                    # FIFO already orders them behind earlier slots' chunks —
                    # a deeper window there releases tail weights a slot
                    # earlier without letting gpsimd-routed future chunks
                    # steal SDMA share from the startup window
                    wj = w_pool.tile([128, WB * DOUT], f8, tag=f"w{j}",
                                     name=f"w{s}_{j}", bufs=4 if j < 2 else 3)
                    if s in WQ:
                        eng = WQ[s][j]
                    else:
                        eng = nc.sync if (s * NWCH + j) % 2 == 0 else nc.scalar
                    eng.dma_start(
                        wj[:], w8[s, :, j * WB * DOUT:(j + 1) * WB * DOUT]
                    )
                    wch.append(wj)
                o_sb = o_pool.tile([128, NN * cap], bf16, tag="o", name=f"o{s}")
                assert cap <= 256
                # k-outer within each n-phase; bank group alternates per
                # phase (0-3 / 4-7), so the banks entering a phase were
                # copied out a full phase earlier and the phase start never
                # waits on the vector engine.
                for p, (n0, n1) in enumerate(((0, 4), (4, 8), (8, 12), (12, NN))):
                    bank0 = 4 * ((s * 4 + p) % 2)
                    for k in range(KC):
                        wk = wch[k // WB]
                        kb = (k % WB) * DOUT
                        for n in range(n0, n1):
                            ps = psums[bank0 + n - n0][:, :cap]
                            nc.tensor.matmul(
                                ps,
                                wk[:, kb + n * 128:kb + (n + 1) * 128],
                                xs[:, k * cap:(k + 1) * cap],
                                start=(k == 0),
                                stop=(k == KC - 1),
                            )
                    for n in range(n0, n1):
                        nc.vector.tensor_copy(
                            o_sb[:, n * cap:(n + 1) * cap],
                            psums[bank0 + n - n0][:, :cap],
                        )
                    # per-phase output DMA shortens the kernel tail; the last
                    # slots' outs ride HWDGE (lower completion latency than
                    # SWDGE on the critical exit path)
                    oeng = nc.sync if s >= len(caps) - 2 else nc.gpsimd
                    oeng.dma_start(
                        out[:, NN * off + n0 * cap:NN * off + n1 * cap],
                        o_sb[:, n0 * cap:n1 * cap],
                    )
    nc.compile()
    return nc


def _run(inputs, trace=False):
    x = np.asarray(inputs["input"], dtype=np.float32)
    w = np.asarray(inputs["weight"], dtype=np.float32)
    counts = np.asarray(inputs["tokens_per_expert"], dtype=np.int64)
    starts = np.concatenate([[0], np.cumsum(counts)[:-1]])

    order = np.argsort(-counts, kind="stable")  # experts by size rank
    perm = list(range(EPC))  # largest-first; deep prefetch covers the tail
    caps = tuple(
        int(np.ceil(max(1, counts[order[r * NCORES:(r + 1) * NCORES]].max()) / 4) * 4)
        for r in perm
    )
    offs = np.concatenate([[0], np.cumsum(caps)]).astype(int)
    S = int(offs[-1])

    if caps not in _cache:
        _cache[caps] = _build(caps)
    nc = _cache[caps]

    # fp8 scale: w*s must fit in e3m4 (max normal 15.5); fold 1/s into x
    s_pow = 2.0 ** np.floor(np.log2(15.49 / np.abs(w).max()))
    x_sc = (x * (1.0 / s_pow)).astype(ml_dtypes.bfloat16)
    w8_full = (w * s_pow).astype(ml_dtypes.float8_e3m4)

    in_maps = []
    for c in range(NCORES):
        xt_pack = np.zeros((128, KC * S), dtype=ml_dtypes.bfloat16)
        w_pack = np.empty((EPC, 128, KC * DOUT), dtype=ml_dtypes.float8_e3m4)
        for s in range(EPC):
            g = int(order[perm[s] * NCORES + c])
            cnt = int(counts[g])
            cap = caps[s]
            o0 = KC * int(offs[s])
            if cnt:
                blk = np.zeros((128, KC, cap), dtype=ml_dtypes.bfloat16)
                blk[:, :, :cnt] = (
                    x_sc[starts[g]:starts[g] + cnt].T
                    .reshape(KC, 128, cnt).transpose(1, 0, 2)
                )
                xt_pack[:, o0:o0 + KC * cap] = blk.reshape(128, KC * cap)
            w_pack[s] = (
                w8_full[g].reshape(KC, 128, DOUT).transpose(1, 0, 2)
                .reshape(128, KC * DOUT)
            )
        in_maps.append({"w8": w_pack, "xt": xt_pack})

    kw = {"trace_cores": list(range(NCORES))} if trace else {}
    res = run_bass_kernel_spmd(nc, in_maps, core_ids=list(range(NCORES)),
                               trace=trace, **kw)

    out = np.empty((T, DOUT), dtype=np.float32)
    for c in range(NCORES):
        ob = res.results[c]["out"]
        for s in range(EPC):
            g = int(order[perm[s] * NCORES + c])
            cnt = int(counts[g])
            cap = caps[s]
            if cnt:
                blk = ob[:, NN * offs[s]:NN * offs[s] + NN * cap]
                blk = blk.reshape(128, NN, cap).transpose(2, 1, 0)
                out[starts[g]:starts[g] + cnt] = (
                    blk.reshape(cap, DOUT)[:cnt].astype(np.float32)
                )
    return out, res


def kernel(**inputs) -> np.ndarray:
    return _run(inputs)[0]


# revision 9
# speedup vs baseline: 1.1148x; 1.0024x over previous
"""Grouped GEMM (MoE routing) Trainium2 kernel — token-streaming fp8 design.

Expert-parallel across 8 NeuronCores with size-sorted slot assignment
(slot s on core c holds the expert of size-rank 8s+c; per-slot capacity
cap_s = roundup4(max count in rank group)).

Key design vs the bf16 token-stationary baseline:
- Weights are quantized to float8e3 (E3M4, 4 mantissa bits) on host with
  a global power-of-2 scale folded into x (y = (x/s) @ (w*s) exactly), so
  weight HBM traffic halves: 68 MB -> 34 MB per core.  x stays bf16 as
  the PE moving operand (mixed-dtype matmul), out written bf16.
- Token-streaming orientation: stationary = w tile [128k x 128n] fp8,
  moving = x^T [128k x cap] bf16, psum [128n x cap].  PE cost scales with
  actual token count instead of ceil(count/128)*128.
- PSUM bank-group rotation: each slot runs 4 n-phases (0,4)(4,8)(8,12)
  (12,13) on alternating bank groups 0-3 / 4-7, so a phase's first matmul
  reuses banks whose psum->sbuf copies completed a full phase earlier —
  no copy-latency bubble at phase/slot boundaries (the bubbles triggered
  the HAM clock governor to halve the PE clock in the small-cap tail).
- Per-phase output DMA keeps the kernel-exit DMA tiny (1-bank phase).
"""
import ml_dtypes
import numpy as np

import concourse.mybir as mybir
import concourse.tile as tile
from concourse import bacc
from concourse.bass_utils import run_bass_kernel_spmd

G, T, DIN, DOUT = 64, 8192, 2560, 1664
NCORES = 8
EPC = G // NCORES   # expert slots per core
KC = DIN // 128     # 20 contraction chunks
NN = DOUT // 128    # 13 output-row chunks

_cache = {}


def _build(caps):
    caps = [int(c) for c in caps if c > 0]
    offs = np.concatenate([[0], np.cumsum(caps)]).astype(int)
    S = int(offs[-1])
    nc = bacc.Bacc(trn_type="TRN2", debug=False)
    f8 = mybir.dt.float8e3
    bf16 = mybir.dt.bfloat16
    f32 = mybir.dt.float32

    # partition-major layouts: every DMA below is a [128, N] slice whose
    # per-partition bytes are contiguous in HBM (large descriptors)
    w8 = nc.dram_tensor("w8", [EPC, 128, KC * DOUT], f8,
                        kind="ExternalInput").ap()
    xt = nc.dram_tensor("xt", [128, KC * S], bf16, kind="ExternalInput").ap()
    out = nc.dram_tensor("out", [128, NN * S], bf16, kind="ExternalOutput").ap()

    WB = 4   # k-chunks per w DMA
    NWCH = KC // WB  # 5 w chunks per slot
    with tile.TileContext(nc) as tc:
        with (
            tc.tile_pool(name="wp", bufs=3) as w_pool,
            tc.tile_pool(name="xp", bufs=1) as x_pool,
            tc.tile_pool(name="op", bufs=2) as o_pool,
            tc.tile_pool(name="ps", bufs=1, space="PSUM") as ps_pool,
        ):
            # PE warm-up: ~6us of dummy matmuls on a zeroed tile so the HAM
            # clock gate reaches 2.4 GHz before the first real matmul, and the
            # PE is busy while the first DMAs land.
            warm_l = x_pool.tile([128, 128], bf16, tag="wl", name="warm_l")
            warm_r = x_pool.tile([128, 512], bf16, tag="wr", name="warm_r")
            nc.vector.memset(warm_l[:], 0)
            nc.vector.memset(warm_r[:], 0)
            pswarm = ps_pool.tile([128, 512], f32, tag="psw", name="pswarm")
            for i in range(36):
                nc.tensor.matmul(pswarm[:], warm_l[:], warm_r[:],
                                 start=True, stop=True)
            psums = {}
            for j in range(7):  # one open accumulation region per bank
                psums[j] = ps_pool.tile([128, 512], f32, tag=f"ps{j}",
                                        name=f"psum{j}")
            psums[7] = pswarm  # warmup bank doubles as the 8th region

            for s, cap in enumerate(caps):
                off = int(offs[s])
                # per-slot x^T tile (1.3MB at cap 256): prefetched like the
                # weights; slots 0/1 ride the fast HWDGE rings so slot 0
                # starts right as the warmup ends, later slots go SWDGE
                xs = x_pool.tile([128, KC * cap], bf16, tag="xs",
                                 name=f"xs{s}", bufs=3)
                # all x^T traffic rides the gpsimd (SWDGE) ring so the two
                # HWDGE rings carry nothing but the weight stream; slot 0's
                # x is split so its head chunks land first
                if s == 0:
                    h = (KC // 2) * cap
                    nc.gpsimd.dma_start(xs[:, :h], xt[:, KC * off:KC * off + h])
                    nc.gpsimd.dma_start(
                        xs[:, h:], xt[:, KC * off + h:KC * (off + cap)]
                    )
                else:
                    nc.gpsimd.dma_start(xs[:], xt[:, KC * off:KC * (off + cap)])
                # 852KB weight chunks strictly alternate the two HWDGE rings
                # (global parity, so consecutive chunks always stream in
                # parallel); the gpsimd ring takes one mid chunk of the tail
                # slots, whose per-slot weight demand rate exceeds what two
                # rings deliver
                wch = []
                for j in range(NWCH):
                    wj = w_pool.tile([128, WB * DOUT], f8, tag=f"w{j}",
                                     name=f"w{s}_{j}", bufs=4 if j < 2 else 3)
                    if s >= 5 and j == 2:
                        eng = nc.gpsimd
                    else:
                        eng = nc.sync if (s * NWCH + j) % 2 == 0 else nc.scalar
                    eng.dma_start(
                        wj[:], w8[s, :, j * WB * DOUT:(j + 1) * WB * DOUT]
                    )
                    wch.append(wj)
                o_sb = o_pool.tile([128, NN * cap], bf16, tag="o", name=f"o{s}")
                assert cap <= 256
                # k-outer within each n-phase.  Big slots (s<5) use two wide
                # phases — their weight chunks are consumed over most of the
                # slot, matching the DMA stream rate.  Small tail slots use
                # four rotated 4-bank phases (groups 0-3 / 4-7 alternating),
                # so a phase's first matmul reuses banks whose psum->sbuf
                # copies completed a full phase earlier — no copy-latency
                # bubble at the boundaries (those bubbles triggered the HAM
                # clock governor to halve the PE clock in the tail).
                if s < 5:
                    phases = ((0, 8, 0), (8, NN, 0))
                else:
                    phases = ((0, 4, 0), (4, 8, 4), (8, 12, 0), (12, NN, 4))
                for n0, n1, bank0 in phases:
                    for k in range(KC):
                        wk = wch[k // WB]
                        kb = (k % WB) * DOUT
                        for n in range(n0, n1):
                            ps = psums[bank0 + n - n0][:, :cap]
                            nc.tensor.matmul(
                                ps,
                                wk[:, kb + n * 128:kb + (n + 1) * 128],
                                xs[:, k * cap:(k + 1) * cap],
                                start=(k == 0),
                                stop=(k == KC - 1),
                            )
                    for n in range(n0, n1):
                        nc.vector.tensor_copy(
                            o_sb[:, n * cap:(n + 1) * cap],
                            psums[bank0 + n - n0][:, :cap],
                        )
                    # per-phase output DMA shortens the kernel tail; the last
                    # slots' outs ride HWDGE (lower completion latency than
                    # SWDGE on the critical exit path); earlier slots
                    # alternate scalar/gpsimd so neither ring bears all outs
                    if s >= len(caps) - 2:
                        oeng = nc.sync
                    else:
                        oeng = nc.scalar if (s + n0 // 4) % 2 else nc.gpsimd
                    oeng.dma_start(
                        out[:, NN * off + n0 * cap:NN * off + n1 * cap],
                        o_sb[:, n0 * cap:n1 * cap],
                    )
    nc.compile()
    return nc


def _run(inputs, trace=False):
    x = np.asarray(inputs["input"], dtype=np.float32)
    w = np.asarray(inputs["weight"], dtype=np.float32)
    counts = np.asarray(inputs["tokens_per_expert"], dtype=np.int64)
    starts = np.concatenate([[0], np.cumsum(counts)[:-1]])

    order = np.argsort(-counts, kind="stable")  # experts by size rank
    perm = list(range(EPC))  # largest-first; deep prefetch covers the tail
    caps = tuple(
        int(np.ceil(max(1, counts[order[r * NCORES:(r + 1) * NCORES]].max()) / 4) * 4)
        for r in perm
    )
    offs = np.concatenate([[0], np.cumsum(caps)]).astype(int)
    S = int(offs[-1])

    if caps not in _cache:
        _cache[caps] = _build(caps)
    nc = _cache[caps]

    # fp8 scale: w*s must fit in e3m4 (max normal 15.5); fold 1/s into x
    s_pow = 2.0 ** np.floor(np.log2(15.49 / np.abs(w).max()))
    x_sc = (x * (1.0 / s_pow)).astype(ml_dtypes.bfloat16)
    w8_full = (w * s_pow).astype(ml_dtypes.float8_e3m4)

    in_maps = []
    for c in range(NCORES):
        xt_pack = np.zeros((128, KC * S), dtype=ml_dtypes.bfloat16)
        w_pack = np.empty((EPC, 128, KC * DOUT), dtype=ml_dtypes.float8_e3m4)
        for s in range(EPC):
            g = int(order[perm[s] * NCORES + c])
            cnt = int(counts[g])
            cap = caps[s]
            o0 = KC * int(offs[s])
            if cnt:
                blk = np.zeros((128, KC, cap), dtype=ml_dtypes.bfloat16)
                blk[:, :, :cnt] = (
                    x_sc[starts[g]:starts[g] + cnt].T
                    .reshape(KC, 128, cnt).transpose(1, 0, 2)
                )
                xt_pack[:, o0:o0 + KC * cap] = blk.reshape(128, KC * cap)
            w_pack[s] = (
                w8_full[g].reshape(KC, 128, DOUT).transpose(1, 0, 2)
                .reshape(128, KC * DOUT)
            )
        in_maps.append({"w8": w_pack, "xt": xt_pack})

    kw = {"trace_cores": list(range(NCORES))} if trace else {}
    res = run_bass_kernel_spmd(nc, in_maps, core_ids=list(range(NCORES)),
                               trace=trace, **kw)

    out = np.empty((T, DOUT), dtype=np.float32)
    for c in range(NCORES):
        ob = res.results[c]["out"]
        for s in range(EPC):
            g = int(order[perm[s] * NCORES + c])
            cnt = int(counts[g])
            cap = caps[s]
            if cnt:
                blk = ob[:, NN * offs[s]:NN * offs[s] + NN * cap]
                blk = blk.reshape(128, NN, cap).transpose(2, 1, 0)
                out[starts[g]:starts[g] + cnt] = (
                    blk.reshape(cap, DOUT)[:cnt].astype(np.float32)
                )
    return out, res


def kernel(**inputs) -> np.ndarray:
    return _run(inputs)[0]


# revision 11
# speedup vs baseline: 1.1203x; 1.0049x over previous
"""Grouped GEMM (MoE routing) Trainium2 kernel — token-streaming fp8 design.

Expert-parallel across 8 NeuronCores with size-sorted slot assignment
(slot s on core c holds the expert of size-rank 8s+c; per-slot capacity
cap_s = roundup4(max count in rank group)).

Key design vs the bf16 token-stationary baseline:
- Weights are quantized to float8e3 (E3M4, 4 mantissa bits) on host with
  a global power-of-2 scale folded into x (y = (x/s) @ (w*s) exactly), so
  weight HBM traffic halves: 68 MB -> 34 MB per core.  x stays bf16 as
  the PE moving operand (mixed-dtype matmul), out written bf16.
- Token-streaming orientation: stationary = w tile [128k x 128n] fp8,
  moving = x^T [128k x cap] bf16, psum [128n x cap].  PE cost scales with
  actual token count instead of ceil(count/128)*128.
- PSUM bank-group rotation: each slot runs 4 n-phases (0,4)(4,8)(8,12)
  (12,13) on alternating bank groups 0-3 / 4-7, so a phase's first matmul
  reuses banks whose psum->sbuf copies completed a full phase earlier —
  no copy-latency bubble at phase/slot boundaries (the bubbles triggered
  the HAM clock governor to halve the PE clock in the small-cap tail).
- Per-phase output DMA keeps the kernel-exit DMA tiny (1-bank phase).
"""
import ml_dtypes
import numpy as np

import concourse.mybir as mybir
import concourse.tile as tile
from concourse import bacc
from concourse.bass_utils import run_bass_kernel_spmd

G, T, DIN, DOUT = 64, 8192, 2560, 1664
NCORES = 8
EPC = G // NCORES   # expert slots per core
KC = DIN // 128     # 20 contraction chunks
NN = DOUT // 128    # 13 output-row chunks

_cache = {}


def _build(caps):
    caps = [int(c) for c in caps if c > 0]
    offs = np.concatenate([[0], np.cumsum(caps)]).astype(int)
    S = int(offs[-1])
    nc = bacc.Bacc(trn_type="TRN2", debug=False)
    f8 = mybir.dt.float8e3
    bf16 = mybir.dt.bfloat16
    f32 = mybir.dt.float32

    # partition-major layouts: every DMA below is a [128, N] slice whose
    # per-partition bytes are contiguous in HBM (large descriptors)
    w8 = nc.dram_tensor("w8", [EPC, 128, KC * DOUT], f8,
                        kind="ExternalInput").ap()
    xt = nc.dram_tensor("xt", [128, KC * S], bf16, kind="ExternalInput").ap()
    out = nc.dram_tensor("out", [128, NN * S], bf16, kind="ExternalOutput").ap()

    WB = 4   # k-chunks per w DMA
    NWCH = KC // WB  # 5 w chunks per slot
    with tile.TileContext(nc) as tc:
        with (
            tc.tile_pool(name="wp", bufs=3) as w_pool,
            tc.tile_pool(name="xp", bufs=1) as x_pool,
            tc.tile_pool(name="op", bufs=2) as o_pool,
            tc.tile_pool(name="ps", bufs=1, space="PSUM") as ps_pool,
        ):
            # PE warm-up: ~6us of dummy matmuls on a zeroed tile so the HAM
            # clock gate reaches 2.4 GHz before the first real matmul, and the
            # PE is busy while the first DMAs land.
            warm_l = x_pool.tile([128, 128], bf16, tag="wl", name="warm_l")
            warm_r = x_pool.tile([128, 512], bf16, tag="wr", name="warm_r")
            nc.vector.memset(warm_l[:], 0)
            nc.vector.memset(warm_r[:], 0)
            pswarm = ps_pool.tile([128, 512], f32, tag="psw", name="pswarm")
            for i in range(30):
                nc.tensor.matmul(pswarm[:], warm_l[:], warm_r[:],
                                 start=True, stop=True)
            psums = {}
            for j in range(7):  # one open accumulation region per bank
                psums[j] = ps_pool.tile([128, 512], f32, tag=f"ps{j}",
                                        name=f"psum{j}")
            psums[7] = pswarm  # warmup bank doubles as the 8th region

            for s, cap in enumerate(caps):
                off = int(offs[s])
                # per-slot x^T tile (1.3MB at cap 256): prefetched like the
                # weights; slots 0/1 ride the fast HWDGE rings so slot 0
                # starts right as the warmup ends, later slots go SWDGE
                xs = x_pool.tile([128, KC * cap], bf16, tag="xs",
                                 name=f"xs{s}", bufs=3)
                # all x^T traffic rides the gpsimd (SWDGE) ring so the two
                # HWDGE rings carry nothing but the weight stream; slot 0's
                # x is split so its head chunks land first
                if s == 0:
                    h = (KC // 2) * cap
                    nc.gpsimd.dma_start(xs[:, :h], xt[:, KC * off:KC * off + h])
                    nc.gpsimd.dma_start(
                        xs[:, h:], xt[:, KC * off + h:KC * (off + cap)]
                    )
                else:
                    nc.gpsimd.dma_start(xs[:], xt[:, KC * off:KC * (off + cap)])
                # 852KB weight chunks strictly alternate the two HWDGE rings
                # (global parity, so consecutive chunks always stream in
                # parallel); the gpsimd ring takes one mid chunk of the tail
                # slots, whose per-slot weight demand rate exceeds what two
                # rings deliver
                wch = []
                for j in range(NWCH):
                    wj = w_pool.tile([128, WB * DOUT], f8, tag=f"w{j}",
                                     name=f"w{s}_{j}", bufs=4 if j < 2 else 3)
                    if s >= 5 and j == 2:
                        eng = nc.gpsimd
                    else:
                        eng = nc.sync if (s * NWCH + j) % 2 == 0 else nc.scalar
                    eng.dma_start(
                        wj[:], w8[s, :, j * WB * DOUT:(j + 1) * WB * DOUT]
                    )
                    wch.append(wj)
                o_sb = o_pool.tile([128, NN * cap], bf16, tag="o", name=f"o{s}")
                assert cap <= 256
                # k-outer within each n-phase.  Big slots (s<5) use two wide
                # phases — their weight chunks are consumed over most of the
                # slot, matching the DMA stream rate.  Small tail slots use
                # four rotated 4-bank phases (groups 0-3 / 4-7 alternating),
                # so a phase's first matmul reuses banks whose psum->sbuf
                # copies completed a full phase earlier — no copy-latency
                # bubble at the boundaries (those bubbles triggered the HAM
                # clock governor to halve the PE clock in the tail).
                if s < 5:
                    phases = ((0, 8, 0), (8, NN, 0))
                else:
                    phases = ((0, 4, 0), (4, 8, 4), (8, 12, 0), (12, NN, 4))
                for n0, n1, bank0 in phases:
                    for k in range(KC):
                        wk = wch[k // WB]
                        kb = (k % WB) * DOUT
                        for n in range(n0, n1):
                            ps = psums[bank0 + n - n0][:, :cap]
                            nc.tensor.matmul(
                                ps,
                                wk[:, kb + n * 128:kb + (n + 1) * 128],
                                xs[:, k * cap:(k + 1) * cap],
                                start=(k == 0),
                                stop=(k == KC - 1),
                            )
                    for n in range(n0, n1):
                        nc.vector.tensor_copy(
                            o_sb[:, n * cap:(n + 1) * cap],
                            psums[bank0 + n - n0][:, :cap],
                        )
                    # per-phase output DMA shortens the kernel tail; the last
                    # slots' outs ride HWDGE (lower completion latency than
                    # SWDGE on the critical exit path).  Outs must stay off
                    # the weight rings: an out dma_start waits on vector
                    # copies, and on a weight ring it would head-of-line
                    # block later slots' weight chunk issues.
                    oeng = nc.sync if s >= len(caps) - 2 else nc.gpsimd
                    oeng.dma_start(
                        out[:, NN * off + n0 * cap:NN * off + n1 * cap],
                        o_sb[:, n0 * cap:n1 * cap],
                    )
    nc.compile()
    return nc


def _run(inputs, trace=False):
    x = np.asarray(inputs["input"], dtype=np.float32)
    w = np.asarray(inputs["weight"], dtype=np.float32)
    counts = np.asarray(inputs["tokens_per_expert"], dtype=np.int64)
    starts = np.concatenate([[0], np.cumsum(counts)[:-1]])

    order = np.argsort(-counts, kind="stable")  # experts by size rank
    perm = list(range(EPC))  # largest-first; deep prefetch covers the tail
    caps = tuple(
        int(np.ceil(max(1, counts[order[r * NCORES:(r + 1) * NCORES]].max()) / 4) * 4)
        for r in perm
    )
    offs = np.concatenate([[0], np.cumsum(caps)]).astype(int)
    S = int(offs[-1])

    if caps not in _cache:
        _cache[caps] = _build(caps)
    nc = _cache[caps]

    # fp8 scale: w*s must fit in e3m4 (max normal 15.5); fold 1/s into x
    s_pow = 2.0 ** np.floor(np.log2(15.49 / np.abs(w).max()))
    x_sc = (x * (1.0 / s_pow)).astype(ml_dtypes.bfloat16)
    w8_full = (w * s_pow).astype(ml_dtypes.float8_e3m4)

    in_maps = []
    for c in range(NCORES):
        xt_pack = np.zeros((128, KC * S), dtype=ml_dtypes.bfloat16)
        w_pack = np.empty((EPC, 128, KC * DOUT), dtype=ml_dtypes.float8_e3m4)
        for s in range(EPC):
            g = int(order[perm[s] * NCORES + c])
            cnt = int(counts[g])
            cap = caps[s]
            o0 = KC * int(offs[s])
            if cnt:
                blk = np.zeros((128, KC, cap), dtype=ml_dtypes.bfloat16)
                blk[:, :, :cnt] = (
                    x_sc[starts[g]:starts[g] + cnt].T
                    .reshape(KC, 128, cnt).transpose(1, 0, 2)
                )
                xt_pack[:, o0:o0 + KC * cap] = blk.reshape(128, KC * cap)
            w_pack[s] = (
                w8_full[g].reshape(KC, 128, DOUT).transpose(1, 0, 2)
                .reshape(128, KC * DOUT)
            )
        in_maps.append({"w8": w_pack, "xt": xt_pack})

    kw = {"trace_cores": list(range(NCORES))} if trace else {}
    res = run_bass_kernel_spmd(nc, in_maps, core_ids=list(range(NCORES)),
                               trace=trace, **kw)

    out = np.empty((T, DOUT), dtype=np.float32)
    for c in range(NCORES):
        ob = res.results[c]["out"]
        for s in range(EPC):
            g = int(order[perm[s] * NCORES + c])
            cnt = int(counts[g])
            cap = caps[s]
            if cnt:
                blk = ob[:, NN * offs[s]:NN * offs[s] + NN * cap]
                blk = blk.reshape(128, NN, cap).transpose(2, 1, 0)
                out[starts[g]:starts[g] + cnt] = (
                    blk.reshape(cap, DOUT)[:cnt].astype(np.float32)
                )
    return out, res


def kernel(**inputs) -> np.ndarray:
    return _run(inputs)[0]
